# revision 6
# baseline (speedup 1.0000x reference)
"""Expert-parallel MoE SwiGLU kernel for 8 Trainium2 NeuronCores.

Problem: N=4096 tokens, top-2 of E=8 experts, H=2048, I=1408, fp32.

Strategy (expert parallel, per the sharding hint):
  - Host-side dispatch: gather each expert's routed tokens (the "all-to-all
    dispatch" step) while sharding the full inputs; core e gets expert e's
    token slab x_e^T [H, C] plus its weight triple (transposed).
  - Device: each core independently computes
        y_e^T = Wd_e @ (silu(Wg_e @ x_e^T) * (Wu_e @ x_e^T))
    entirely in [feature, token] layout so no on-device transposes are
    needed; matmuls run as fp32r (FP22 single-pass) on the PE.
  - Host-side combine: weighted scatter-add of per-expert outputs back to
    the [N, H] output (the "all-to-all combine" step).

All shapes/sharding are hardcoded for this problem instance; capacity C
(max tokens routed to one expert, padded to a multiple of 128) is computed
from the actual routing at call time and the NEFF is compiled per C
(cached within the process).
"""

import numpy as np

import concourse.bass as bass
import concourse.tile as tile
from concourse import bacc, mybir
from concourse import bass_utils

N, K, E, H, I = 4096, 2, 8, 2048, 1408
P = 128
HCH = H // P   # 16 chunks over hidden dim
ICH = I // P   # 11 chunks over intermediate dim
F32 = mybir.dt.float32
F32R = mybir.dt.float32r


def _chunks(C):
    """Split C (multiple of 128) into free-dim chunks, each a multiple of 128
    in [256, 512] (fp32r matmul needs moving dim >= 256 for full rate)."""
    assert C % P == 0 and C >= 2 * P
    k = C // P
    n = (k + 3) // 4  # number of chunks, each <= 512
    if k < 2 * n:     # guarantee every chunk >= 256
        n = k // 2
    base, rem = divmod(k, n)
    sizes = [(base + (1 if i < rem else 0)) * P for i in range(n)]
    out, off = [], 0
    for s in sizes:
        out.append((off, s))
        off += s
    assert off == C and all(s >= 256 and s <= 512 for _, s in out)
    return out


def _build(C):
    """Build + compile the per-core SwiGLU kernel for capacity C."""
    ch = _chunks(C)
    nc = bacc.Bacc("TRN2", target_bir_lowering=False, debug=False, num_devices=E)

    xT = nc.dram_tensor("xT", [H, C], F32R, kind="ExternalInput")
    wgT = nc.dram_tensor("wgT", [H, I], F32R, kind="ExternalInput")
    wuT = nc.dram_tensor("wuT", [H, I], F32R, kind="ExternalInput")
    wdT = nc.dram_tensor("wdT", [I, H], F32R, kind="ExternalInput")
    outT = nc.dram_tensor("outT", [H, C], F32, kind="ExternalOutput")

    x_r = xT.ap().rearrange("(ho p) c -> p ho c", p=P)      # [128, 16, C]
    wg_r = wgT.ap().rearrange("(ho p) i -> p ho i", p=P)    # [128, 16, I]
    wu_r = wuT.ap().rearrange("(ho p) i -> p ho i", p=P)
    wd_r = wdT.ap().rearrange("(io p) h -> p io h", p=P)    # [128, 11, H]
    out_r = outT.ap().rearrange("(ho p) c -> p ho c", p=P)  # [128, 16, C]

    with tile.TileContext(nc) as tc:
        with (
            tc.tile_pool(name="xpool", bufs=1) as xpool,
            tc.tile_pool(name="hpool", bufs=1) as hpool,
            tc.tile_pool(name="wpool", bufs=3) as wpool,
            tc.tile_pool(name="dpool", bufs=3) as dpool,
            tc.tile_pool(name="opool", bufs=2) as opool,
        ):
            # resident activations: x^T and hidden^T
            # x streams on the SP HWDGE ring; weights go via gpsimd SWDGE so
            # the two never serialize behind each other, and the first
            # weight chunks land within ~1us so the PE starts immediately.
            x_sb = xpool.tile([P, HCH, C], F32R)
            for h in range(HCH):
                nc.sync.dma_start(x_sb[:, h, :], x_r[:, h, :])
            hid_sb = hpool.tile([P, ICH, C], F32R)

            # ---- phase 1: gate/up projections + SwiGLU -> hidden^T [I, C]
            with tc.tile_pool(name="ps1", bufs=1, space="PSUM") as ps1:
                for i in range(ICH):
                    w_sb = wpool.tile([P, 2, HCH, P], F32R, tag="w12")
                    isl = slice(i * P, (i + 1) * P)
                    if i == 0:
                        # fine-grained so h=0 weights arrive first
                        nc.gpsimd.dma_start(w_sb[:, 0, 0:2], wg_r[:, 0:2, isl])
                        nc.gpsimd.dma_start(w_sb[:, 0, 2:8], wg_r[:, 2:8, isl])
                        nc.gpsimd.dma_start(w_sb[:, 0, 8:16], wg_r[:, 8:16, isl])
                        nc.gpsimd.dma_start(w_sb[:, 1, 0:8], wu_r[:, 0:8, isl])
                        nc.gpsimd.dma_start(w_sb[:, 1, 8:16], wu_r[:, 8:16, isl])
                    else:
                        nc.gpsimd.dma_start(w_sb[:, 0], wg_r[:, :, isl])
                        nc.gpsimd.dma_start(w_sb[:, 1], wu_r[:, :, isl])
                    ps_g = [
                        ps1.tile([P, cw], F32, name=f"psg_{i}_{n}", tag=f"psg{n}")
                        for n, (c0, cw) in enumerate(ch)
                    ]
                    ps_u = [
                        ps1.tile([P, cw], F32, name=f"psu_{i}_{n}", tag=f"psu{n}")
                        for n, (c0, cw) in enumerate(ch)
                    ]
                    for m, ps in ((0, ps_g), (1, ps_u)):
                        for h in range(HCH):
                            lhsT = w_sb[:, m, h, :]
                            for n, (c0, cw) in enumerate(ch):
                                nc.tensor.matmul(
                                    ps[n][:],
                                    lhsT,
                                    x_sb[:, h, c0:c0 + cw],
                                    start=(h == 0),
                                    stop=(h == HCH - 1),
                                )
                    for n, (c0, cw) in enumerate(ch):
                        hs = hid_sb[:, i, c0:c0 + cw]
                        nc.scalar.activation(
                            out=hs, in_=ps_g[n][:],
                            func=mybir.ActivationFunctionType.Silu,
                        )
                        nc.vector.tensor_mul(out=hs, in0=hs, in1=ps_u[n][:])

            # ---- phase 2: down projection -> out^T [H, C]
            with tc.tile_pool(name="ps2", bufs=2, space="PSUM") as ps2:
                for h in range(HCH):
                    wd_sb = dpool.tile([P, ICH, P], F32R, tag="wd")
                    nc.gpsimd.dma_start(wd_sb[:], wd_r[:, :, h * P:(h + 1) * P])
                    ps_d = [
                        ps2.tile([P, cw], F32, name=f"psd_{h}_{n}", tag=f"psd{n}")
                        for n, (c0, cw) in enumerate(ch)
                    ]
                    for i in range(ICH):
                        lhsT = wd_sb[:, i, :]
                        for n, (c0, cw) in enumerate(ch):
                            nc.tensor.matmul(
                                ps_d[n][:],
                                lhsT,
                                hid_sb[:, i, c0:c0 + cw],
                                start=(i == 0),
                                stop=(i == ICH - 1),
                            )
                    o_sb = opool.tile([P, C], F32, tag="o")
                    for n, (c0, cw) in enumerate(ch):
                        nc.vector.tensor_copy(o_sb[:, c0:c0 + cw], ps_d[n][:])
                    nc.sync.dma_start(out_r[:, h, :], o_sb[:])

    nc.compile()
    return nc


_NC_CACHE = {}


def _get_nc(C):
    if C not in _NC_CACHE:
        _NC_CACHE[C] = _build(C)
    return _NC_CACHE[C]


def kernel(x, topk_ids, topk_weight, Wg, Wu, Wd):
    x = np.asarray(x, dtype=np.float32)
    topk_ids = np.asarray(topk_ids)
    topk_weight = np.asarray(topk_weight, dtype=np.float32)

    # ---- host-side dispatch (the all-to-all by topk_ids)
    flat = topk_ids.reshape(-1).astype(np.int64)
    order = np.argsort(flat, kind="stable")
    counts = np.bincount(flat, minlength=E)
    toks = order // K          # token index per sorted slot
    ks = order % K             # which of the top-k slots
    bounds = np.cumsum(counts)
    starts = bounds - counts

    C = max(2 * P, int(-(-counts.max() // P)) * P)
    nc = _get_nc(C)

    in_maps = []
    tok_e, k_e = [], []
    for e in range(E):
        te = toks[starts[e]:bounds[e]]
        ke = ks[starts[e]:bounds[e]]
        tok_e.append(te)
        k_e.append(ke)
        xT_e = np.zeros((H, C), np.float32)
        xT_e[:, :len(te)] = x[te].T
        in_maps.append({
            "xT": xT_e,
            "wgT": np.ascontiguousarray(np.asarray(Wg[e], np.float32).T),
            "wuT": np.ascontiguousarray(np.asarray(Wu[e], np.float32).T),
            "wdT": np.ascontiguousarray(np.asarray(Wd[e], np.float32).T),
        })

    res = bass_utils.run_bass_kernel_spmd(nc, in_maps, core_ids=list(range(E)))

    # ---- host-side combine (weighted scatter-add)
    out = np.zeros((N, H), np.float32)
    for e in range(E):
        te, ke = tok_e[e], k_e[e]
        if len(te) == 0:
            continue
        yT = res.results[e]["outT"][:, :len(te)]          # [H, count]
        w = topk_weight[te, ke].astype(np.float32)
        out[te] += (yT * w[None, :]).T
    return out
